# revision 4
# baseline (speedup 1.0000x reference)
import sys

sys.path.insert(0, "/opt/trn_rl_repo")
import numpy as np
import ml_dtypes

import concourse.bass as bass
import concourse.mybir as mybir
from concourse import bacc
from concourse.bass import ds
from concourse.tile import TileContext

# ---- model constants (hardcoded per spec) ----
LAGS = np.array([1, 2, 3, 4, 5, 6, 7, 14, 21, 28])
MAX_LAG = 28
RING = 29  # ring buffer slots (mod-29 so a 29-step window has no collision)
N_LAGS = 10
HID = 512
BATCH, CTX, HOR = 128, 720, 168
NDEC = HOR - 1  # 167 decode steps
NT = CTX + NDEC  # 887 outputs
N_CORES = 8
BPC = BATCH // N_CORES  # 16 batch per core
EMB_ROWS = 12
# x rows: tgt/prev, 10 lags, logscale, 12 one-hot cat (emb table and bias
# folded into the one-hot weight columns)
KDIM = 24

F32 = mybir.dt.float32
BF16 = mybir.dt.bfloat16
AF = mybir.ActivationFunctionType
ALU = mybir.AluOpType

_BF = ml_dtypes.bfloat16

# sharded-weight column counts (per core)
W0C, W1C, B1C = 5 * 2048 // N_CORES, 8 * 2048 // N_CORES, 256 // N_CORES
# int8 weight transport: dequant scale per (input-dim partition, gate-group
# of 256 cols). w0: 5 k-tiles * 8 groups = 40 scales; w1: 64.
NSC0, NSC1 = 5 * 8, 8 * 8
# f32 shared param: b1 (32 cols/core) + w0 scales (5) + w1 scales (8)
FSHC = B1C + NSC0 // N_CORES + NSC1 // N_CORES
SERN = CTX + MAX_LAG  # 748


def _ring_slot(k: int) -> int:
    return (-k) % RING


def _runs_inc(slots, stride):
    """Group slot list into maximal runs with the given +stride; returns
    list of (start_idx_in_list, src_start_slot, n)."""
    runs = []
    i = 0
    while i < len(slots):
        j = i
        while (j + 1 < len(slots)
               and slots[j + 1] == slots[j] + stride):
            j += 1
        runs.append((i, slots[i], j - i + 1))
        i = j + 1
    return runs


def _build_device_program(b_head_val: float, use_collective: bool = True,
                          n_ctx: int = CTX, n_dec: int = NDEC,
                          unroll_ctx: bool = False):
    nc = bacc.Bacc("TRN2", target_bir_lowering=False, debug=False,
                   num_devices=N_CORES)

    # ---- external inputs (consolidated into 3 params for transfer speed) ----
    I8 = mybir.dt.int8
    # int8 blob cols: w0 shard | w1 shard | cats (14208 = 128*111, row-major)
    CATC = (NT * BPC + BPC) // 128  # 111
    i8p = nc.declare_dram_parameter("iblob", [128, W0C + W1C + CATC], I8,
                                    isOutput=False)
    fsp = nc.declare_dram_parameter("fsh", [128, FSHC], F32, isOutput=False)
    # bf16 flat blob: wh (128*4) | ring0 (31*16) | sert (748*16)
    BF_WH, BF_RG, BF_SER = 0, 512, 1008
    bfp = nc.declare_dram_parameter("bblob", [1, 1008 + SERN * BPC], BF16,
                                    isOutput=False)
    yo = nc.declare_dram_parameter("y", [1, NT * BPC], BF16, isOutput=True)

    # collectives may not read IO tensors -> bounce via Internal DRAM
    w0b = nc.dram_tensor("w0b", [128, W0C], I8, kind="Internal")
    w1b = nc.dram_tensor("w1b", [128, W1C], I8, kind="Internal")
    fsb = nc.dram_tensor("fsb", [128, FSHC], F32, kind="Internal")
    w0g = nc.dram_tensor("w0g", [N_CORES * 128, W0C], I8, kind="Internal",
                         addr_space="Shared")
    w1g = nc.dram_tensor("w1g", [N_CORES * 128, W1C], I8, kind="Internal",
                         addr_space="Shared")
    fsg = nc.dram_tensor("fsg", [N_CORES * 128, FSHC], F32, kind="Internal",
                         addr_space="Shared")

    with TileContext(nc) as tc:
        with (
            tc.tile_pool(name="wpool", bufs=1) as wpool,
            tc.tile_pool(name="state", bufs=1) as state,
            tc.tile_pool(name="work", bufs=2) as work,
            tc.tile_pool(name="psum", bufs=2, space="PSUM") as ppool,
        ):
            # resident weights/features
            w0s = wpool.tile([128, 5 * 2048], BF16, tag="w0s")
            w1s = wpool.tile([128, 8 * 2048], BF16, tag="w1s")
            whs = wpool.tile([128, 4], BF16, tag="whs")
            b1s = wpool.tile([128, 256], F32, tag="b1s")
            w0sc = wpool.tile([128, NSC0], F32, tag="w0sc")
            w1sc = wpool.tile([128, NSC1], F32, tag="w1sc")
            w0q = wpool.tile([128, 5 * 2048], mybir.dt.int8, tag="w0q")
            w1q = wpool.tile([128, 8 * 2048], mybir.dt.int8, tag="w1q")
            xcs = wpool.tile([KDIM, CTX * BPC], BF16, tag="xcs")
            # xds rows 0..11 = decode one-hot, row 12 = logscale
            xds = wpool.tile([13, NDEC * BPC], BF16, tag="xds")
            ohc = wpool.tile([EMB_ROWS, CTX * BPC], BF16, tag="ohc")
            catb = wpool.tile([EMB_ROWS, CTX * BPC], I8, tag="catb")
            catall = wpool.tile([1, NT * BPC + BPC], I8, tag="catall")
            iot = wpool.tile([EMB_ROWS, 1], F32, tag="iot")

            nc.gpsimd.dma_start(w0b[:], i8p[:, 0:W0C])
            nc.gpsimd.dma_start(w1b[:], i8p[:, W0C:W0C + W1C])
            nc.gpsimd.dma_start(fsb[:], fsp[:])
            rg = [list(range(N_CORES))]
            nc.gpsimd.collective_compute(
                "AllGather", ALU.bypass, replica_groups=rg,
                ins=[w0b[:]], outs=[w0g[:]])
            nc.gpsimd.collective_compute(
                "AllGather", ALU.bypass, replica_groups=rg,
                ins=[w1b[:]], outs=[w1g[:]])
            nc.gpsimd.collective_compute(
                "AllGather", ALU.bypass, replica_groups=rg,
                ins=[fsb[:]], outs=[fsg[:]])
            NS0, NS1 = NSC0 // N_CORES, NSC1 // N_CORES
            for c in range(N_CORES):
                nc.sync.dma_start(w0q[:, c * W0C:(c + 1) * W0C],
                                  w0g[c * 128:(c + 1) * 128, :])
                nc.sync.dma_start(w1q[:, c * W1C:(c + 1) * W1C],
                                  w1g[c * 128:(c + 1) * 128, :])
                nc.sync.dma_start(b1s[:, c * B1C:(c + 1) * B1C],
                                  fsg[c * 128:(c + 1) * 128, 0:B1C])
                nc.sync.dma_start(w0sc[:, c * NS0:(c + 1) * NS0],
                                  fsg[c * 128:(c + 1) * 128, B1C:B1C + NS0])
                nc.sync.dma_start(w1sc[:, c * NS1:(c + 1) * NS1],
                                  fsg[c * 128:(c + 1) * 128,
                                      B1C + NS0:B1C + NS0 + NS1])
            # dequant int8 -> bf16 with per-(partition, 256-col group) scales
            for k in range(5):
                for g in range(8):
                    c0 = k * 2048 + g * 256
                    eng = nc.vector if (k * 8 + g) % 2 == 0 else nc.gpsimd
                    eng.tensor_scalar_mul(
                        w0s[:, c0:c0 + 256], w0q[:, c0:c0 + 256],
                        w0sc[:, k * 8 + g:k * 8 + g + 1])
            for k in range(8):
                for g in range(8):
                    c0 = k * 2048 + g * 256
                    eng = nc.vector if (k * 8 + g) % 2 == 0 else nc.gpsimd
                    eng.tensor_scalar_mul(
                        w1s[:, c0:c0 + 256], w1q[:, c0:c0 + 256],
                        w1sc[:, k * 8 + g:k * 8 + g + 1])
            nc.sync.dma_start(whs[:], bfp[0:1, BF_WH:BF_WH + 512])
            nc.sync.dma_start(catall[0:1, :],
                              i8p[:, W0C + W1C:W0C + W1C + CATC])
            # xcs rows 0..10 (tgt + 10 lags) are shifted reads of sert
            for i, l in enumerate([0] + [int(v) for v in LAGS]):
                nc.scalar.dma_start(
                    xcs[i:i + 1, :],
                    bfp[0:1, BF_SER + (MAX_LAG - l) * BPC:
                        BF_SER + (SERN - l) * BPC])
            # row 11 logscale: stride-0 broadcast from ring0 row 30
            nc.scalar.dma_start(
                xcs[11:12, :],
                bfp[0:1, BF_RG + (RING + 1) * BPC:BF_RG + (RING + 2) * BPC]
                .unsqueeze(1).broadcast_to((1, CTX, BPC)))
            # iota column: iot[p, 0] = p
            nc.gpsimd.iota(iot[:, 0:1], pattern=[[0, 1]], base=0,
                           channel_multiplier=1,
                           allow_small_or_imprecise_dtypes=True)
            # context one-hot: broadcast cat row to 12 partitions, compare
            # against iota, then DMA into xcs rows 12..23
            nc.gpsimd.partition_broadcast(catb[:], catall[0:1, 0:CTX * BPC])
            nc.vector.tensor_scalar(ohc[:], catb[:], iot[:, 0:1], None,
                                    ALU.is_equal)
            nc.scalar.dma_start(xcs[12:KDIM, :], ohc[:])
            # decode one-hot directly into xds rows 0..11; logscale row 12
            nc.scalar.dma_start(
                xds[12:13, :],
                bfp[0:1, BF_RG + (RING + 1) * BPC:BF_RG + (RING + 2) * BPC]
                .unsqueeze(1).broadcast_to((1, NDEC, BPC)))
            nc.gpsimd.partition_broadcast(
                catb[:, 0:NDEC * BPC],
                catall[0:1, CTX * BPC:CTX * BPC + NDEC * BPC])
            nc.vector.tensor_scalar(xds[0:EMB_ROWS, :],
                                    catb[:, 0:NDEC * BPC], iot[:, 0:1], None,
                                    ALU.is_equal)

            # persistent state
            h0 = state.tile([128, 64], BF16, tag="h0")
            c0 = state.tile([128, 64], F32, tag="c0")
            h1 = state.tile([128, 64], BF16, tag="h1")
            c1 = state.tile([128, 64], F32, tag="c1")
            uxA = state.tile([KDIM, BPC], BF16, tag="uxA")
            uxB = state.tile([KDIM, BPC], BF16, tag="uxB")
            ring = state.tile([RING, BPC], BF16, tag="ring")
            yprev = state.tile([1, BPC], BF16, tag="yprev")
            ysb = state.tile([1, NT * BPC], BF16, tag="ysb")

            for t in (h0, c0, h1, c1):
                nc.gpsimd.memset(t[:], 0.0)
            # ux row 11 = logscale, constant across decode steps
            nc.sync.dma_start(
                uxA[11:12, :],
                bfp[0:1, BF_RG + (RING + 1) * BPC:BF_RG + (RING + 2) * BPC])
            nc.sync.dma_start(
                uxB[11:12, :],
                bfp[0:1, BF_RG + (RING + 1) * BPC:BF_RG + (RING + 2) * BPC])
            nc.sync.dma_start(ring[:],
                              bfp[0:1, BF_RG:BF_RG + RING * BPC])

            def nonlin(psum, h, c, add_bias):
                # gate order i,f,o,g: sigmoid [0:192], tanh [192:256]
                if add_bias is not None:
                    nc.vector.tensor_tensor(psum[:], psum[:], add_bias[:],
                                            ALU.add)
                sg = work.tile([128, 192], F32, tag="sg")
                tg = work.tile([128, 64], F32, tag="tg")
                t1 = work.tile([128, 64], F32, tag="t1")
                tcc = work.tile([128, 64], F32, tag="tcc")
                nc.scalar.activation(sg[:], psum[:, 0:192], AF.Sigmoid)
                nc.scalar.activation(tg[:], psum[:, 192:256], AF.Tanh)
                nc.vector.tensor_tensor(t1[:], sg[:, 0:64], tg[:], ALU.mult)
                nc.vector.tensor_tensor(c[:], sg[:, 64:128], c[:], ALU.mult)
                nc.vector.tensor_tensor(c[:], c[:], t1[:], ALU.add)
                nc.scalar.activation(tcc[:], c[:], AF.Tanh)
                nc.vector.tensor_tensor(h[:], sg[:, 128:192], tcc[:], ALU.mult)

            def layer1(xsrc, x_first):
                """L1 gates: psum += W_hh0 @ h0 (k=0..3) + W_ih0x @ x (k=4)."""
                ps0 = ppool.tile([128, 256], F32, tag="ps0")
                korder = [4, 0, 1, 2, 3] if x_first else [0, 1, 2, 3, 4]
                for m in range(16):
                    for ki, k in enumerate(korder):
                        if k == 4:
                            rhs = xsrc
                            lhsT = w0s[0:KDIM,
                                       k * 2048 + m * 128:k * 2048 + (m + 1) * 128]
                        else:
                            rhs = h0[:, k * BPC:(k + 1) * BPC]
                            lhsT = w0s[:, k * 2048 + m * 128:k * 2048 + (m + 1) * 128]
                        nc.tensor.matmul(
                            ps0[:, m * BPC:(m + 1) * BPC],
                            lhsT=lhsT,
                            rhs=rhs,
                            start=(ki == 0), stop=(ki == 4),
                        )
                nonlin(ps0, h0, c0, None)

            def layer2():
                """L2 gates: psum += W_hh1 @ h1 (k=4..7) first, then
                W_ih1 @ h0_new (k=0..3)."""
                ps1 = ppool.tile([128, 256], F32, tag="ps1")
                korder = [4, 5, 6, 7, 0, 1, 2, 3]
                for m in range(16):
                    for ki, k in enumerate(korder):
                        rhs = (h1[:, (k - 4) * BPC:(k - 4 + 1) * BPC] if k >= 4
                               else h0[:, k * BPC:(k + 1) * BPC])
                        nc.tensor.matmul(
                            ps1[:, m * BPC:(m + 1) * BPC],
                            lhsT=w1s[:, k * 2048 + m * 128:k * 2048 + (m + 1) * 128],
                            rhs=rhs,
                            start=(ki == 0), stop=(ki == 7),
                        )
                nonlin(ps1, h1, c1, b1s)

            def head(ycol, write_yprev):
                psy = ppool.tile([128, BPC], F32, tag="psy")
                for k in range(4):
                    nc.tensor.matmul(
                        psy[0:1, :], lhsT=whs[:, k:k + 1],
                        rhs=h1[:, k * BPC:(k + 1) * BPC],
                        start=(k == 0), stop=(k == 3),
                    )
                nc.scalar.copy(ysb[0:1, ycol], psy[0:1, :])
                if write_yprev:
                    nc.scalar.activation(yprev[0:1, :], psy[0:1, :], AF.Copy,
                                         bias=b_head_val)

            def ctx_tick(i):
                layer1(xcs[:, ds(i * BPC, BPC)], x_first=True)
                layer2()
                head(ds(i * BPC, BPC), write_yprev=False)

            if unroll_ctx:
                for i in range(n_ctx):
                    ctx_tick(i)
            else:
                with tc.For_i(0, n_ctx, 1,
                              hint_engines=(mybir.EngineType.PE,)) as i:
                    ctx_tick(i)

            # context epilogue: y at t = n_ctx-1 feeds decode (z_{-1})
            if n_dec > 0:
                psy = ppool.tile([128, BPC], F32, tag="psy")
                for k in range(4):
                    nc.tensor.matmul(
                        psy[0:1, :], lhsT=whs[:, k:k + 1],
                        rhs=h1[:, k * BPC:(k + 1) * BPC],
                        start=(k == 0), stop=(k == 3),
                    )
                nc.scalar.activation(yprev[0:1, :], psy[0:1, :], AF.Copy,
                                     bias=b_head_val)
                nc.sync.dma_start(ring[_ring_slot(-1):_ring_slot(-1) + 1, :],
                                  yprev[0:1, :])

            def dec_tick(s):
                ux = uxA if s % 2 == 0 else uxB
                scol = s * BPC
                # ---- off-critical-path prefetches ----
                if s >= 1:
                    w = _ring_slot(s - 1)
                    nc.sync.dma_start(ring[w:w + 1, :], yprev[0:1, :])
                # features: logscale + emb
                nc.gpsimd.dma_start(ux[12:KDIM, :],
                                    xds[0:EMB_ROWS, ds(scol, BPC)])
                # lags 1..7 -> ux rows 1..7 (slot (l+1-s) mod 29, ascending)
                slots_a = [(int(l) + 1 - s) % RING for l in LAGS[:7]]
                for off, srd, n in _runs_inc(slots_a, 1):
                    nc.sync.dma_start(ux[1 + off:1 + off + n, :],
                                      ring[srd:srd + n, :])
                # lags 14,21,28 -> ux rows 8..10 (stride 7)
                slots_b = [(int(l) + 1 - s) % RING for l in LAGS[7:]]
                for off, srd, n in _runs_inc(slots_b, 7):
                    if n == 1:
                        nc.sync.dma_start(ux[8 + off:8 + off + 1, :],
                                          ring[srd:srd + 1, :])
                    else:
                        nc.sync.dma_start(ux[8 + off:8 + off + n, :],
                                          ring[srd:srd + 7 * (n - 1) + 1:7, :])
                # ---- critical path: y_s -> ux[0] -> L1 x-matmul ----
                nc.vector.tensor_copy(ux[0:1, :], yprev[0:1, :])
                layer1(ux[:, :], x_first=False)
                layer2()
                head(ds(CTX * BPC + scol, BPC), write_yprev=True)

            for s in range(n_dec):
                dec_tick(s)

            nc.sync.dma_start(yo[:], ysb[:])

    nc.compile()
    return nc


# torch gate order i,f,g,o -> device order i,f,o,g
_GATE_PERM = np.concatenate([
    np.arange(0, 512), np.arange(512, 1024),
    np.arange(1536, 2048), np.arange(1024, 1536)])


def _wt_layout(Wcat, nk):
    """Wcat [2048, K] -> [128, nk*2048]; out[p, k*2048+g] = Wcat[g, k*128+p]."""
    f = np.float32
    K = Wcat.shape[1]
    Wp = np.zeros((2048, nk * 128), f)
    Wp[:, :K] = Wcat
    out = np.zeros((128, nk * 2048), f)
    for k in range(nk):
        out[:, k * 2048:(k + 1) * 2048] = Wp[:, k * 128:(k + 1) * 128].T
    return out


def _quant_i8(wf, nk):
    """wf [128, nk*2048] f32 -> (int8 same shape, scales [128, nk*8] f32).
    Scale per (partition, 256-col gate group)."""
    q = np.zeros(wf.shape, np.int8)
    sc = np.zeros((128, nk * 8), np.float32)
    for k in range(nk):
        for g in range(8):
            c0 = k * 2048 + g * 256
            blk = wf[:, c0:c0 + 256]
            s = np.maximum(np.abs(blk).max(axis=1) / 127.0, 1e-20)
            q[:, c0:c0 + 256] = np.clip(
                np.round(blk / s[:, None]), -127, 127).astype(np.int8)
            sc[:, k * 8 + g] = s
    return q, sc


def _host_prep(X, pad_mask, emb, W_ih0, W_hh0, b_ih0, b_hh0,
               W_ih1, W_hh1, b_ih1, b_hh1, W_head, b_head):
    f = np.float32
    X = np.asarray(X, f).copy()
    X[:, -HOR:, 0] = 0.0
    past = X[:, :CTX + MAX_LAG, 0][:, ::-1]
    Xt = X[:, MAX_LAG:]
    mask = np.asarray(pad_mask)[:, MAX_LAG:][:, :CTX].astype(f)
    scale = (np.abs(Xt[:, :CTX, 0]) * mask).sum(1) / np.clip(mask.sum(1), 1.0, None)
    scale = np.maximum(scale, 1e-10).astype(f)
    tgt = Xt[:, :, 0] / scale[:, None]
    past_s = past / scale[:, None]
    idx = (CTX - 1 - np.arange(CTX))[:, None] + LAGS[None, :]
    lags_ctx = past_s[:, idx]  # [B, C, 10]
    logscale = np.log(scale)
    cat = Xt[:, :, 1].astype(np.int32)
    seq_emb = np.asarray(emb, f)[cat]  # [B, C+H, 5]

    # category index rows (0..11) for context and decode positions
    cat_ctx = cat[:, :CTX].astype(f)          # [B, CTX]
    cat_dec = cat[:, CTX:CTX + NDEC].astype(f)

    # weights with gate perm; emb table and b0 folded into 12 one-hot cols
    b0 = (np.asarray(b_ih0, f) + np.asarray(b_hh0, f))[_GATE_PERM]
    b1 = (np.asarray(b_ih1, f) + np.asarray(b_hh1, f))[_GATE_PERM]
    Wih0 = np.asarray(W_ih0, f)[_GATE_PERM]
    Whh0 = np.asarray(W_hh0, f)[_GATE_PERM]
    Wih1 = np.asarray(W_ih1, f)[_GATE_PERM]
    Whh1 = np.asarray(W_hh1, f)[_GATE_PERM]
    # one-hot cols: W_emb @ emb[j] + b0  (emb feature weights are cols 12..16)
    Woh = Wih0[:, 12:17] @ np.asarray(emb, f).T + b0[:, None]  # [2048, 12]
    Wih0x = np.concatenate([Wih0[:, 0:12], Woh], 1)  # [2048, 24]
    w0f = _wt_layout(np.concatenate([Whh0, Wih0x], 1), 5)
    w1f = _wt_layout(np.concatenate([Wih1, Whh1], 1), 8)
    w0q, w0sc = _quant_i8(w0f, 5)
    w1q, w1sc = _quant_i8(w1f, 8)
    whn = np.zeros((128, 4), f)
    for k in range(4):
        whn[:, k] = np.asarray(W_head, f)[0, k * 128:(k + 1) * 128]
    whn = whn.astype(_BF)

    b1f = np.zeros((128, 256), f)
    g = b1.reshape(16, 128)
    for m in range(16):
        b1f[:, m * BPC:(m + 1) * BPC] = g[m][:, None]
    # shared f32 param: b1 | w0 scales | w1 scales, column-sharded
    fsh = np.concatenate([b1f, w0sc, w1sc], axis=1)  # [128, 256+40+64]

    bh = float(np.asarray(b_head, f).reshape(-1)[0])

    NS0, NS1 = NSC0 // N_CORES, NSC1 // N_CORES
    in_maps = []
    for cidx in range(N_CORES):
        sl = slice(cidx * BPC, (cidx + 1) * BPC)
        # per-core shard of fsh in device gather order: b1 | w0sc | w1sc
        fshc = np.concatenate([
            b1f[:, cidx * B1C:(cidx + 1) * B1C],
            w0sc[:, cidx * NS0:(cidx + 1) * NS0],
            w1sc[:, cidx * NS1:(cidx + 1) * NS1]], axis=1)
        # cat indices: cats[t*16+b] = cat_ctx[b, t], then decode positions
        catm = np.concatenate([cat_ctx[sl].T.reshape(-1),
                               cat_dec[sl].T.reshape(-1),
                               np.zeros(BPC, f)]).astype(np.uint8)[None, :]
        # sert[j, b] = past_s[b, 747-j] (reversed, time-major)
        sert = past_s[sl][:, ::-1].T.copy()  # [748, 16]
        # ring0: slot 0 = past[27], slot 1 unused (z_-1), slots 2..28 = past[0..26]
        # row 29 unused; row 30 = per-batch logscale; row 31 = iota 0..11
        ring0 = np.zeros((RING + 2, BPC), f)
        ring0[0] = past_s[sl, 27].T
        for s in range(2, RING):
            ring0[s] = past_s[sl, s - 2].T
        ring0[RING + 1] = logscale[sl]
        iblob = np.concatenate([
            w0q[:, cidx * W0C:(cidx + 1) * W0C],
            w1q[:, cidx * W1C:(cidx + 1) * W1C],
            catm.astype(np.int8).reshape(128, -1)], axis=1)
        bblob = np.concatenate([
            whn.reshape(-1),
            ring0.astype(_BF).reshape(-1),
            sert.astype(_BF).reshape(-1)])[None, :]
        in_maps.append({
            "iblob": np.ascontiguousarray(iblob),
            "fsh": np.ascontiguousarray(fshc),
            "bblob": np.ascontiguousarray(bblob),
        })
    return in_maps, scale, bh


class _Runner:
    """Persistent-jit PJRT runner (mirrors run_bass_via_pjrt, reusable)."""

    def __init__(self, nc):
        import jax
        from jax.sharding import Mesh, PartitionSpec
        from jax.experimental.shard_map import shard_map
        from concourse import bass2jax
        from concourse.bass2jax import _bass_exec_p, install_neuronx_cc_hook

        install_neuronx_cc_hook()
        self.jax = jax
        self.nc = nc
        partition_name = (nc.partition_id_tensor.name
                          if nc.partition_id_tensor else None)
        in_names, out_names, out_avals, zero_shapes = [], [], [], []
        for alloc in nc.m.functions[0].allocations:
            if not isinstance(alloc, mybir.MemoryLocationSet):
                continue
            name = alloc.memorylocations[0].name
            if alloc.kind == "ExternalInput":
                if name != partition_name:
                    in_names.append(name)
            elif alloc.kind == "ExternalOutput":
                shape = tuple(alloc.tensor_shape)
                dtype = mybir.dt.np(alloc.dtype)
                out_names.append(name)
                out_avals.append(jax.core.ShapedArray(shape, dtype))
                zero_shapes.append((shape, dtype))
        self.in_names, self.out_names = in_names, out_names
        self.out_avals, self.zero_shapes = out_avals, zero_shapes
        n_params, n_outs = len(in_names), len(out_names)
        in_names_full = in_names + out_names
        if partition_name is not None:
            in_names_full.append(partition_name)
        donate = tuple(range(n_params, n_params + n_outs))

        def _body(*args):
            operands = list(args)
            if partition_name is not None:
                operands.append(bass2jax.partition_id_tensor())
            outs = _bass_exec_p.bind(
                *operands,
                out_avals=tuple(out_avals),
                in_names=tuple(in_names_full),
                out_names=tuple(out_names),
                lowering_input_output_aliases=(),
                sim_require_finite=True,
                sim_require_nnan=True,
                nc=nc,
            )
            return tuple(outs)

        devices = jax.devices()[:N_CORES]
        mesh = Mesh(np.asarray(devices), ("core",))
        in_specs = (PartitionSpec("core"),) * (n_params + n_outs)
        out_specs = (PartitionSpec("core"),) * n_outs
        self.fn = jax.jit(
            shard_map(_body, mesh=mesh, in_specs=in_specs,
                      out_specs=out_specs, check_rep=False),
            donate_argnums=donate, keep_unused=True,
        )

    def run(self, in_maps):
        np_ = np
        concat_in = [
            np_.concatenate([np_.asarray(in_maps[c][name])
                             for c in range(N_CORES)], axis=0)
            for name in self.in_names
        ]
        concat_zeros = [np_.zeros((N_CORES * s[0], *s[1:]), d)
                        for s, d in self.zero_shapes]
        out_arrs = self.fn(*concat_in, *concat_zeros)
        outs = []
        for i, name in enumerate(self.out_names):
            full = np_.asarray(out_arrs[i])
            outs.append(full.reshape(N_CORES, *self.out_avals[i].shape))
        return {name: outs[i] for i, name in enumerate(self.out_names)}


LAST_EXEC_NS = None


def kernel(X, pad_mask, emb, W_ih0, W_hh0, b_ih0, b_hh0,
           W_ih1, W_hh1, b_ih1, b_hh1, W_head, b_head, H, context_length):
    global LAST_EXEC_NS
    in_maps, scale, bh = _host_prep(
        X, pad_mask, emb, W_ih0, W_hh0, b_ih0, b_hh0,
        W_ih1, W_hh1, b_ih1, b_hh1, W_head, b_head)
    nc = _build_device_program(bh)
    runner = _Runner(nc)
    import time as _time
    for attempt in range(3):
        try:
            runner.run(in_maps)  # warm: trace + compile + first exec
            break
        except Exception:
            if attempt == 2:
                raise
            _time.sleep(2.0)  # transient device error: retry
    _t = _time.time()
    try:
        res = runner.run(in_maps)  # timed: upload + exec + fetch
    except Exception:
        _t = _time.time()
        res = runner.run(in_maps)
    LAST_EXEC_NS = (_time.time() - _t) * 1e9
    y_all = res["y"]  # [8, 1, NT*BPC] bf16
    ys = []
    for cidx in range(N_CORES):
        arr = y_all[cidx].reshape(NT, BPC).astype(np.float32)
        ys.append(arr.T)
    y = np.concatenate(ys, 0)
    y = (y + bh) * scale[:, None]
    return y[:, :, None].astype(np.float32)
